# revision 25
# baseline (speedup 1.0000x reference)
"""Trainium2 Bass kernel for nn_DirectMultiStepModel (2-layer graph-GRU + big Linear + softmax).

Self-contained: takes FULL inputs, shards nodes across 8 NeuronCores internally,
runs a single SPMD NEFF with on-device collectives, returns the FULL (1, 100) output.

Design (v2):
  Phase A: all 24 GRU1 steps, feature-major, fp8-DoubleRow gate matmuls
    (x paired with h0, h_n as (h0,h1) pair; packed DR weights, x16 weight
    scaling rescaled at activation). h1 staged node-major per step via
    DMA-engine transpose (no PE transposes) + fp8 cast; AllGather every
    TB=4 steps (6 blocks, fully hidden behind GRU1 compute).
  Phase B: 24 units of dense aggregation (M^T resident fp8 x16; gathered
    h fp8 stationary; DR matmuls, free-dim {512,512,256}) fused with GRU2
    steps (a-pair DR + bf16 h-single).
  Tail: h2 AllGather -> agg2 -> final linear as 640 DR pair-matmuls over
    4 parallel PSUM accumulation chains (fp8 lin_W x1024, streamed in
    chunks) -> AllReduce -> softmax.
"""
import sys
import types
import numpy as np
import ml_dtypes

import concourse.bass as bass
import concourse.bacc as bacc
import concourse.mybir as mybir
import concourse.tile as tile
from concourse.bass_utils import run_bass_kernel_spmd

BF16 = ml_dtypes.bfloat16
E4M3 = ml_dtypes.float8_e4m3
F32 = mybir.dt.float32
BF = mybir.dt.bfloat16
F8 = mybir.dt.float8e4
P = 128
DRM = mybir.MatmulPerfMode.DoubleRow
WS = 16.0      # weight scale for fp8 gate weights
AS = 8.0       # activation (a1 / out2) fp8 scale
MS = 16.0      # M (adjacency) fp8 scale
LS = 1024.0    # lin_W fp8 scale
OPAD = 112     # padded out dim for lin DR stride alignment


def _install_ntff_hook():
    """Register the NTFF profile hook the agent image's antenv lacks (no-op if present)."""
    try:
        import antenv.axon_hooks  # noqa: F401
        return
    except ImportError:
        pass
    try:
        import trn_agent_boot.trn_boot as tb
        hooks = types.ModuleType("antenv.axon_hooks")
        _h = [None]
        hooks.set_axon_ntff_profile_hook = lambda h: _h.__setitem__(0, h)
        hooks.get_axon_ntff_profile_hook = lambda: _h[0]
        sys.modules["antenv.axon_hooks"] = hooks
        import antenv
        antenv.axon_hooks = hooks
        hook = tb._ntff_profile_via_ctypes('/opt/axon/libaxon_pjrt.so')
        if hook is not None:
            hooks.set_axon_ntff_profile_hook(hook)
    except Exception:
        pass


class Cfg:
    def __init__(self, T=24, N=10000, DIN=128, H1=256, H2=128, OUT=100, NC=8, TB=4):
        self.T, self.N, self.DIN, self.H1, self.H2, self.OUT, self.NC = T, N, DIN, H1, H2, OUT, NC
        self.NOWN = -(-N // (NC * P)) * P          # per-core padded node count (1280)
        self.NPAD = self.NOWN * NC                 # total padded nodes (10240)
        self.NT = self.NOWN // P                   # own node tiles (10)
        self.CT = self.NPAD // P                   # contraction tiles (80)
        self.PS1 = H1 // P                         # 2
        self.PS2 = H2 // P                         # 1
        self.TB = TB                               # timesteps per AllGather block
        assert T % TB == 0
        self.BLKS = [2, 2, 4, 4, 4, 4, 4]          # per-AllGather block sizes
        assert sum(self.BLKS) == T
        self.NB = len(self.BLKS)
        self.BOFF = [sum(self.BLKS[:i]) for i in range(self.NB)]
        self.NPAIR = self.NOWN // 2                # 640 node pairs for final linear


CHUNKS = ((0, 512), (512, 384), (896, 384))


def build(cfg: Cfg):
    c = cfg
    nc = bacc.Bacc("TRN2", target_bir_lowering=False, debug=False, num_devices=c.NC)

    # ---- kernel I/O ----
    xT = nc.dram_tensor("xT", [c.T, P, c.NOWN], F8, kind="ExternalInput").ap()
    mTt = nc.dram_tensor("mTt", [P, c.CT * c.NOWN], F8, kind="ExternalInput").ap()
    wrz1 = nc.dram_tensor("wrz1", [P, 4 * 2 * P], F8, kind="ExternalInput").ap()
    wrz1h = nc.dram_tensor("wrz1h", [P, 4 * P], BF, kind="ExternalInput").ap()
    wnx1 = nc.dram_tensor("wnx1", [P, 2 * P], F8, kind="ExternalInput").ap()
    wnh1 = nc.dram_tensor("wnh1", [P, 2 * 2 * P], F8, kind="ExternalInput").ap()
    wrz2 = nc.dram_tensor("wrz2", [P, 2 * 2 * P], F8, kind="ExternalInput").ap()
    wrz2h = nc.dram_tensor("wrz2h", [P, 2 * P], BF, kind="ExternalInput").ap()
    wn2 = nc.dram_tensor("wn2", [P, 2 * P], F8, kind="ExternalInput").ap()
    wnh2 = nc.dram_tensor("wnh2", [P, P], BF, kind="ExternalInput").ap()
    b_rz1 = nc.dram_tensor("b_rz1", [P, 4], F32, kind="ExternalInput").ap()
    b_in1 = nc.dram_tensor("b_in1", [P, 2], F32, kind="ExternalInput").ap()
    b_hn1 = nc.dram_tensor("b_hn1", [P, 2], F32, kind="ExternalInput").ap()
    b_rz2 = nc.dram_tensor("b_rz2", [P, 2], F32, kind="ExternalInput").ap()
    b_in2 = nc.dram_tensor("b_in2", [P, 1], F32, kind="ExternalInput").ap()
    b_hn2 = nc.dram_tensor("b_hn2", [P, 1], F32, kind="ExternalInput").ap()
    cb1x8 = nc.dram_tensor("cb1x8", [P, 2], F32, kind="ExternalInput").ap()
    cb2x8 = nc.dram_tensor("cb2x8", [P, 1], F32, kind="ExternalInput").ap()
    linb = nc.dram_tensor("linb", [1, c.OUT], F32, kind="ExternalInput").ap()
    lwp = nc.dram_tensor("lwp", [P, c.NPAIR * 2 * OPAD], F8, kind="ExternalInput").ap()
    out = nc.dram_tensor("out", [1, c.OUT], F32, kind="ExternalOutput").ap()

    rg = [list(range(c.NC))]
    AGRS = [b * c.PS1 * P for b in c.BLKS]   # payload rows per core per block
    NOWN = c.NOWN
    t2b = {}
    for tb, (off, sz) in enumerate(zip(c.BOFF, c.BLKS)):
        for tt in range(sz):
            t2b[off + tt] = (tb, tt)

    Sig = mybir.ActivationFunctionType.Sigmoid
    Tanh = mybir.ActivationFunctionType.Tanh
    Iden = mybir.ActivationFunctionType.Identity
    Relu = mybir.ActivationFunctionType.Relu
    Exp = mybir.ActivationFunctionType.Exp
    Copy = mybir.ActivationFunctionType.Copy
    Mult = mybir.AluOpType.mult
    Add = mybir.AluOpType.add

    with tile.TileContext(nc) as tc:
        with tc.tile_pool(name="dram", bufs=1, space="DRAM") as dram:
            ag_ins = [dram.tile([AGRS[i], NOWN], F8, name=f"ag_in{i}")
                      for i in range(c.NB)]
            ag_outs = [dram.tile([AGRS[i] * c.NC, NOWN], F8, addr_space="Shared",
                                 name=f"ag_out{i}") for i in range(c.NB)]
            ag2_in = dram.tile([P, NOWN], F8)
            ag2_out = dram.tile([P * c.NC, NOWN], F8, addr_space="Shared")
            ar_in = dram.tile([1, c.OUT], F32)
            ar_out = dram.tile([1, c.OUT], F32, addr_space="Shared")

            with tc.tile_pool(name="const", bufs=1) as cpool, \
                 tc.tile_pool(name="mtp", bufs=1) as mtp:
                # ---- constants ----
                wrz1_sb = cpool.tile([P, 4 * 2 * P], F8)
                nc.sync.dma_start(wrz1_sb[:], wrz1[:])
                wrz1h_sb = cpool.tile([P, 4 * P], BF)
                nc.sync.dma_start(wrz1h_sb[:], wrz1h[:])
                wnx1_sb = cpool.tile([P, 2 * P], F8)
                nc.sync.dma_start(wnx1_sb[:], wnx1[:])
                wnh1_sb = cpool.tile([P, 2 * 2 * P], F8)
                nc.sync.dma_start(wnh1_sb[:], wnh1[:])
                wrz2_sb = cpool.tile([P, 2 * 2 * P], F8)
                nc.sync.dma_start(wrz2_sb[:], wrz2[:])
                wrz2h_sb = cpool.tile([P, 2 * P], BF)
                nc.sync.dma_start(wrz2h_sb[:], wrz2h[:])
                wn2_sb = cpool.tile([P, 2 * P], F8)
                nc.sync.dma_start(wn2_sb[:], wn2[:])
                wnh2_sb = cpool.tile([P, P], BF)
                nc.sync.dma_start(wnh2_sb[:], wnh2[:])

                def load_bias(src, k):
                    t = cpool.tile([P, k], F32, name=f"b_{src.tensor.name}")
                    nc.sync.dma_start(t[:], src[:])
                    return t
                brz1_sb = load_bias(b_rz1, 4)
                bin1_sb = load_bias(b_in1, 2)
                bhn1_sb = load_bias(b_hn1, 2)
                brz2_sb = load_bias(b_rz2, 2)
                bin2_sb = load_bias(b_in2, 1)
                bhn2_sb = load_bias(b_hn2, 1)
                cb1_sb = load_bias(cb1x8, 2)
                cb2_sb = load_bias(cb2x8, 1)
                linb_sb = cpool.tile([1, c.OUT], F32)
                nc.sync.dma_start(linb_sb[:], linb[:])

                # M^T resident in SBUF (fp8 x16), loaded in 4 chunks
                mtq = mtp.tile([P, c.CT * NOWN], F8)
                QC = c.CT * NOWN // 4
                for q in range(4):
                    nc.sync.dma_start(mtq[:, q * QC:(q + 1) * QC],
                                      mTt[:, q * QC:(q + 1) * QC])
                mt3 = mtq[:].rearrange("p (ct n) -> p ct n", n=NOWN)

                w4_rz1 = wrz1_sb[:].rearrange("p (g r m) -> p g r m", g=4, r=2)
                w3_rz1h = wrz1h_sb[:].rearrange("p (g m) -> p g m", g=4)
                w3_nx1 = wnx1_sb[:].rearrange("p (g m) -> p g m", g=2)
                w4_nh1 = wnh1_sb[:].rearrange("p (g r m) -> p g r m", g=2, r=2)
                w4_rz2 = wrz2_sb[:].rearrange("p (g r m) -> p g r m", g=2, r=2)
                w3_rz2h = wrz2h_sb[:].rearrange("p (g m) -> p g m", g=2)

                # ===== merged pipeline: GRU1 step t | agg+GRU2 unit t-LAG =====
                LAG = 2 * c.TB
                with tc.tile_pool(name="pA", bufs=1) as pA:
                  h1 = pA.tile([P, 2 * NOWN], BF)
                  nc.vector.memset(h1[:], 0.0)
                  h2 = pA.tile([P, NOWN], BF)
                  nc.vector.memset(h2[:], 0.0)
                  xh0 = pA.tile([P, 3 * NOWN], F8)
                  xh1 = pA.tile([P, 3 * NOWN], F8)
                  xhb = [xh0, xh1]
                  nc.vector.memset(xh0[:, NOWN:3 * NOWN], 0.0)
                  nc.sync.dma_start(xh0[:, 0:NOWN], xT[0])
                  with tc.tile_pool(name="pAw", bufs=1) as pAw, \
                       tc.tile_pool(name="pAs", bufs=1) as pAs, \
                       tc.tile_pool(name="pAs2", bufs=2) as pAs2, \
                       tc.tile_pool(name="hstp", bufs=2) as hstp, \
                       tc.tile_pool(name="a1p", bufs=2) as a1p, \
                       tc.tile_pool(name="pBw", bufs=1) as pBw, \
                       tc.tile_pool(name="psA", bufs=3, space="PSUM") as psA, \
                       tc.tile_pool(name="psB", bufs=1, space="PSUM") as psB, \
                       tc.tile_pool(name="psG", bufs=2, space="PSUM") as psG:
                    for it in range(c.T + LAG):
                      if it < c.T:
                        t = it
                        xh = xhb[t % 2]
                        xh_next = xhb[(t + 1) % 2]
                        if t + 1 < c.T:
                            nc.sync.dma_start(xh_next[:, 0:NOWN], xT[t + 1])
                        xh3 = xh[:].rearrange("p (r n) -> p r n", r=3)
                        rzsb = pAw.tile([P, 4 * NOWN], BF, tag="rzsb")
                        insb = pAw.tile([P, 2 * NOWN], BF, tag="insb")
                        hnsb = pAw.tile([P, 2 * NOWN], BF, tag="hnsb")
                        for (co, fl) in CHUNKS:
                            # r,z gates: DR(x, h0_fp8) + single(h1 bf16)
                            for g in range(4):
                                pg = psA.tile([P, fl], F32, tag="pg")
                                nc.tensor.matmul(pg[:], w4_rz1[:, g], xh3[:, 0:2, co:co + fl],
                                                 start=True, stop=False, perf_mode=DRM)
                                nc.tensor.matmul(pg[:], w3_rz1h[:, g],
                                                 h1[:, NOWN + co:NOWN + co + fl],
                                                 start=False, stop=True)
                                nc.scalar.activation(rzsb[:, g * NOWN + co:g * NOWN + co + fl],
                                                     pg[:], Sig, bias=brz1_sb[:, g:g + 1],
                                                     scale=1.0 / WS)
                            # i_n: x part only (single fp8); rescale+bias on scalar
                            for g2 in range(2):
                                pi = psA.tile([P, fl], F32, tag="pg")
                                nc.tensor.matmul(pi[:], w3_nx1[:, g2], xh[:, co:co + fl],
                                                 start=True, stop=True)
                                nc.vector.tensor_scalar(
                                    insb[:, g2 * NOWN + co:g2 * NOWN + co + fl], pi[:],
                                    1.0 / WS, bin1_sb[:, g2:g2 + 1], Mult, Add)
                            # h_n: DR(h0, h1) fp8; rescale+bias on vector
                            for g2 in range(2):
                                ph = psA.tile([P, fl], F32, tag="pg")
                                nc.tensor.matmul(ph[:], w4_nh1[:, g2], xh3[:, 1:3, co:co + fl],
                                                 start=True, stop=True, perf_mode=DRM)
                                nc.vector.tensor_scalar(
                                    hnsb[:, g2 * NOWN + co:g2 * NOWN + co + fl], ph[:],
                                    1.0 / WS, bhn1_sb[:, g2:g2 + 1], Mult, Add)
                        # gate math (full width): n = tanh(i_n + r*hn); h' = n + z*(h-n)
                        for g2 in range(2):
                            sl = slice(g2 * NOWN, (g2 + 1) * NOWN)
                            z_sl = slice((2 + g2) * NOWN, (3 + g2) * NOWN)
                            nc.vector.tensor_mul(hnsb[:, sl], rzsb[:, sl], hnsb[:, sl])
                            nc.vector.tensor_add(hnsb[:, sl], hnsb[:, sl], insb[:, sl])
                            nc.scalar.activation(insb[:, sl], hnsb[:, sl], Tanh)
                            nc.gpsimd.tensor_sub(hnsb[:, sl], h1[:, sl], insb[:, sl])
                            nc.vector.tensor_mul(hnsb[:, sl], rzsb[:, z_sl], hnsb[:, sl])
                            nc.vector.tensor_add(h1[:, sl], insb[:, sl], hnsb[:, sl])
                            nc.vector.tensor_copy(
                                xh_next[:, NOWN + g2 * NOWN:NOWN + (g2 + 1) * NOWN],
                                h1[:, sl])
                        # stage h1_t node-major fp8 (single transpose + single DMA)
                        tb, tt = t2b[t]
                        stgT = pAs.tile([P, 2 * c.NT, P], BF, tag="stgT")
                        nc.sync.dma_start_transpose(stgT[:], h1[:])
                        stg8 = pAs2.tile([P, 2 * NOWN], F8, tag="stg8")
                        nc.vector.tensor_copy(
                            stg8[:].rearrange("p (a b) -> p a b", b=P), stgT[:])
                        ro = tt * 2 * P
                        nc.sync.dma_start(
                            ag_ins[tb][ro:ro + 2 * P, :].rearrange(
                                "(s p) n -> p s n", s=2),
                            stg8[:].rearrange("p (s n) -> p s n", s=2))
                        if tt == c.BLKS[tb] - 1:
                            nc.gpsimd.collective_compute(
                                "AllGather", mybir.AluOpType.bypass, replica_groups=rg,
                                ins=[ag_ins[tb].opt()], outs=[ag_outs[tb].opt()])
                      if it >= LAG:
                        u = it - LAG
                        ub, ut = t2b[u]
                        a1h = a1p.tile([P, 2 * NOWN], F8, tag="a1h")
                        for ps in range(2):
                            hst = hstp.tile([P, c.CT * P], F8, tag=f"hst{ps}",
                                            name=f"hst{ps}")
                            for r in range(c.NC):
                                ro = r * AGRS[ub] + (ut * 2 + ps) * P
                                nc.sync.dma_start(hst[:, r * NOWN:(r + 1) * NOWN],
                                                  ag_outs[ub][ro:ro + P, :])
                            hst3 = hst[:].rearrange("p (ct f) -> p ct f", f=P)
                            pas = []
                            for ci in range(3):
                                pa = psB.tile([P, CHUNKS[ci][1]], F32, tag=f"pa{ci}",
                                              name=f"pa{ci}")
                                pas.append(pa)
                            for cp in range(c.CT // 2):
                                for ci, (co, fl) in enumerate(CHUNKS):
                                    nc.tensor.matmul(
                                        pas[ci][:], hst3[:, 2 * cp:2 * cp + 2, :],
                                        mt3[:, 2 * cp:2 * cp + 2, co:co + fl],
                                        start=(cp == 0), stop=(cp == c.CT // 2 - 1),
                                        perf_mode=DRM)
                            for ci, (co, fl) in enumerate(CHUNKS):
                                nc.scalar.activation(
                                    a1h[:, ps * NOWN + co:ps * NOWN + co + fl], pas[ci][:],
                                    Relu, bias=cb1_sb[:, ps:ps + 1], scale=AS / MS)
                        # ---- GRU2 step u ----
                        a3 = a1h[:].rearrange("p (r n) -> p r n", r=2)
                        rz2 = pBw.tile([P, 2 * NOWN], BF, tag="rz2")
                        in2 = pBw.tile([P, NOWN], BF, tag="in2")
                        hn2 = pBw.tile([P, NOWN], BF, tag="hn2")
                        for (co, fl) in CHUNKS:
                            for g in range(2):
                                pg = psG.tile([P, fl], F32, tag="pg2")
                                nc.tensor.matmul(pg[:], w4_rz2[:, g], a3[:, :, co:co + fl],
                                                 start=True, stop=False, perf_mode=DRM)
                                nc.tensor.matmul(pg[:], w3_rz2h[:, g], h2[:, co:co + fl],
                                                 start=False, stop=True)
                                nc.scalar.activation(rz2[:, g * NOWN + co:g * NOWN + co + fl],
                                                     pg[:], Sig, bias=brz2_sb[:, g:g + 1],
                                                     scale=1.0 / WS)
                            pi = psG.tile([P, fl], F32, tag="pg2")
                            nc.tensor.matmul(pi[:], wn2_sb[:].rearrange("p (r m) -> p r m", r=2),
                                             a3[:, :, co:co + fl],
                                             start=True, stop=True, perf_mode=DRM)
                            nc.vector.tensor_scalar(in2[:, co:co + fl], pi[:],
                                                    1.0 / WS, bin2_sb[:, 0:1], Mult, Add)
                            ph = psG.tile([P, fl], F32, tag="pg2")
                            nc.tensor.matmul(ph[:], wnh2_sb[:], h2[:, co:co + fl],
                                             start=True, stop=True)
                            nc.vector.tensor_scalar(hn2[:, co:co + fl], ph[:],
                                                    1.0, bhn2_sb[:, 0:1], Mult, Add)
                        sl = slice(0, NOWN)
                        z_sl = slice(NOWN, 2 * NOWN)
                        nc.vector.tensor_mul(hn2[:, sl], rz2[:, sl], hn2[:, sl])
                        nc.vector.tensor_add(hn2[:, sl], hn2[:, sl], in2[:, sl])
                        nc.scalar.activation(in2[:, sl], hn2[:, sl], Tanh)
                        nc.gpsimd.tensor_sub(hn2[:, sl], h2[:, sl], in2[:, sl])
                        nc.vector.tensor_mul(hn2[:, sl], rz2[:, z_sl], hn2[:, sl])
                        nc.vector.tensor_add(h2[:, sl], in2[:, sl], hn2[:, sl])

                  # ---- stage h2 node-major fp8, AllGather ----
                  if True:
                    with tc.tile_pool(name="p2s", bufs=1) as p2s:
                        stg2T = p2s.tile([P, c.NT, P], BF)
                        nc.sync.dma_start_transpose(stg2T[:], h2[:])
                        stg28 = p2s.tile([P, NOWN], F8)
                        nc.vector.tensor_copy(
                            stg28[:].rearrange("p (a b) -> p a b", b=P), stg2T[:])
                        nc.sync.dma_start(ag2_in[:], stg28[:])
                        nc.gpsimd.collective_compute(
                            "AllGather", mybir.AluOpType.bypass, replica_groups=rg,
                            ins=[ag2_in.opt()], outs=[ag2_out.opt()])

                    # ---- tail: agg2 + linear + softmax ----
                    with tc.tile_pool(name="p4", bufs=1) as p4, \
                         tc.tile_pool(name="p4w", bufs=2) as p4w, \
                         tc.tile_pool(name="psT", bufs=1, space="PSUM") as psT, \
                         tc.tile_pool(name="psL", bufs=1, space="PSUM") as psL:
                        LC = 80                      # pairs per lw chunk
                        CW = LC * 2 * OPAD

                        def lwc_dma(cc):
                            t = p4w.tile([P, CW], F8, tag="lwc", name="lwc")
                            q = CW // 4
                            for k in range(4):
                                nc.sync.dma_start(
                                    t[:, k * q:(k + 1) * q],
                                    lwp[:, cc * CW + k * q:cc * CW + (k + 1) * q])
                            return t
                        lw_next = lwc_dma(0)   # prefetch during AG2
                        h2g = p4.tile([P, c.CT * P], F8)
                        for r in range(c.NC):
                            nc.sync.dma_start(h2g[:, r * NOWN:(r + 1) * NOWN],
                                              ag2_out[r * P:(r + 1) * P, :])
                        h2g3 = h2g[:].rearrange("p (ct f) -> p ct f", f=P)
                        out2T = p4.tile([P, NOWN], F8)
                        pas2 = []
                        for ci in range(3):
                            pa2 = psT.tile([P, CHUNKS[ci][1]], F32, tag=f"pa2{ci}",
                                           name=f"pa2{ci}")
                            pas2.append(pa2)
                        for cp in range(c.CT // 2):
                            for ci, (co, fl) in enumerate(CHUNKS):
                                nc.tensor.matmul(
                                    pas2[ci][:], h2g3[:, 2 * cp:2 * cp + 2, :],
                                    mt3[:, 2 * cp:2 * cp + 2, co:co + fl],
                                    start=(cp == 0), stop=(cp == c.CT // 2 - 1),
                                    perf_mode=DRM)
                        for ci, (co, fl) in enumerate(CHUNKS):
                            nc.scalar.activation(out2T[:, co:co + fl], pas2[ci][:],
                                                 Iden, bias=cb2_sb[:, 0:1], scale=AS / MS)
                        # final linear: 640 DR pair-matmuls, 4 psum chains
                        o3 = out2T[:].rearrange("p (r n) -> p r n", r=2)
                        plogs = []
                        for ch in range(4):
                            pl = psL.tile([1, 512], F32, tag=f"pl{ch}", name=f"pl{ch}")
                            plogs.append(pl)
                        for cc in range(c.NPAIR // LC):
                            lwc = lw_next
                            if cc + 1 < c.NPAIR // LC:
                                lw_next = lwc_dma(cc + 1)
                            lw4 = lwc[:].rearrange("p (j r o) -> p j r o", r=2, o=OPAD)
                            for jj in range(LC):
                                j = cc * LC + jj
                                ch = j % 4
                                nc.tensor.matmul(plogs[ch][:, 0:OPAD], o3[:, :, j:j + 1],
                                                 lw4[:, jj],
                                                 start=(j < 4), stop=(j >= c.NPAIR - 4),
                                                 perf_mode=DRM)
                        lpart = p4.tile([1, c.OUT], F32)
                        nc.vector.tensor_copy(lpart[:], plogs[0][:, 0:c.OUT])
                        for ch in range(1, 4):
                            nc.vector.tensor_add(lpart[:], lpart[:], plogs[ch][:, 0:c.OUT])
                        nc.sync.dma_start(ar_in[:], lpart[:])
                        nc.gpsimd.collective_compute(
                            "AllReduce", mybir.AluOpType.add, replica_groups=rg,
                            ins=[ar_in.opt()], outs=[ar_out.opt()])
                        lg = p4.tile([1, c.OUT], F32)
                        nc.sync.dma_start(lg[:], ar_out[:])
                        nc.vector.tensor_scalar_mul(lg[:], lg[:], 1.0 / (AS * LS))
                        nc.vector.tensor_add(lg[:], lg[:], linb_sb[:])
                        mx = p4.tile([1, 1], F32)
                        nc.vector.tensor_reduce(mx[:], lg[:], mybir.AxisListType.X,
                                                mybir.AluOpType.max, negate=True)
                        ex = p4.tile([1, c.OUT], F32)
                        nc.scalar.activation(ex[:], lg[:], Exp, bias=mx[:, 0:1])
                        sm = p4.tile([1, 1], F32)
                        nc.vector.tensor_reduce(sm[:], ex[:], mybir.AxisListType.X,
                                                mybir.AluOpType.add)
                        rcp = p4.tile([1, 1], F32)
                        nc.vector.reciprocal(rcp[:], sm[:])
                        res = p4.tile([1, c.OUT], F32)
                        nc.vector.tensor_scalar_mul(res[:], ex[:], rcp[:, 0:1])
                        nc.sync.dma_start(out[:], res[:])

    nc.compile()
    return nc


def host_prep(cfg: Cfg, x, edge_index, W_ih1, W_hh1, b_ih1, b_hh1, bias1,
              W_ih2, W_hh2, b_ih2, b_hh2, bias2, lin_W, lin_b):
    c = cfg
    x = np.asarray(x, np.float32)
    edge_index = np.asarray(edge_index)
    row, col = edge_index[0], edge_index[1]
    loops = np.arange(c.N, dtype=row.dtype)
    row = np.concatenate([row, loops])
    col = np.concatenate([col, loops])
    deg = np.zeros(c.N, np.float32)
    np.add.at(deg, col, 1.0)
    dis = np.where(deg > 0, deg ** -0.5, 0.0).astype(np.float32)
    norm = dis[row] * dis[col]
    M = np.zeros((c.NPAD, c.NPAD), np.float32)
    np.add.at(M, (col, row), norm)

    xp = np.zeros((c.T, c.NPAD, c.DIN), np.float32)
    xp[:, :c.N, :] = x

    W_ih1 = np.asarray(W_ih1, np.float32); W_hh1 = np.asarray(W_hh1, np.float32)
    W_ih2 = np.asarray(W_ih2, np.float32); W_hh2 = np.asarray(W_hh2, np.float32)
    b_ih1 = np.asarray(b_ih1, np.float32); b_hh1 = np.asarray(b_hh1, np.float32)
    b_ih2 = np.asarray(b_ih2, np.float32); b_hh2 = np.asarray(b_hh2, np.float32)
    lin_W = np.asarray(lin_W, np.float32)

    Wih1T = W_ih1.T            # (128, 768)
    Whh1T = W_hh1.T            # (256, 768)
    Wih2T = W_ih2.T            # (256, 384)
    Whh2T = W_hh2.T            # (128, 384)
    H1, H2 = c.H1, c.H2

    wrz1 = np.zeros((P, 4, 2, P), np.float32)
    wrz1h = np.zeros((P, 4, P), np.float32)
    for g in range(4):
        cs = slice(g * P, (g + 1) * P)
        wrz1[:, g, 0, :] = WS * Wih1T[:, cs]
        wrz1[:, g, 1, :] = WS * Whh1T[0:P, cs]
        wrz1h[:, g, :] = WS * Whh1T[P:2 * P, cs]
    wnx1 = np.zeros((P, 2, P), np.float32)
    wnh1 = np.zeros((P, 2, 2, P), np.float32)
    for g2 in range(2):
        cs = slice(2 * H1 + g2 * P, 2 * H1 + (g2 + 1) * P)
        wnx1[:, g2, :] = WS * Wih1T[:, cs]
        wnh1[:, g2, 0, :] = WS * Whh1T[0:P, cs]
        wnh1[:, g2, 1, :] = WS * Whh1T[P:2 * P, cs]
    wrz2 = np.zeros((P, 2, 2, P), np.float32)
    wrz2h = np.zeros((P, 2, P), np.float32)
    for g in range(2):
        cs = slice(g * P, (g + 1) * P)
        wrz2[:, g, 0, :] = (WS / AS) * Wih2T[0:P, cs]
        wrz2[:, g, 1, :] = (WS / AS) * Wih2T[P:2 * P, cs]
        wrz2h[:, g, :] = WS * Whh2T[:, cs]
    wn2 = np.zeros((P, 2, P), np.float32)
    cs = slice(2 * H2, 3 * H2)
    wn2[:, 0, :] = (WS / AS) * Wih2T[0:P, cs]
    wn2[:, 1, :] = (WS / AS) * Wih2T[P:2 * P, cs]
    wnh2 = Whh2T[:, cs]

    def cols(v, k):
        return np.asarray(v, np.float32).reshape(k, P).T.copy()  # [P, k]

    common = dict(
        wrz1=wrz1.reshape(P, -1).astype(E4M3),
        wrz1h=wrz1h.reshape(P, -1).astype(BF16),
        wnx1=wnx1.reshape(P, -1).astype(E4M3),
        wnh1=wnh1.reshape(P, -1).astype(E4M3),
        wrz2=wrz2.reshape(P, -1).astype(E4M3),
        wrz2h=wrz2h.reshape(P, -1).astype(BF16),
        wn2=wn2.reshape(P, -1).astype(E4M3),
        wnh2=wnh2.astype(BF16),
        b_rz1=cols((b_ih1 + b_hh1)[:2 * H1], 4),
        b_in1=cols(b_ih1[2 * H1:], 2),
        b_hn1=cols(b_hh1[2 * H1:], 2),
        b_rz2=cols((b_ih2 + b_hh2)[:2 * H2], 2),
        b_in2=cols(b_ih2[2 * H2:], 1),
        b_hn2=cols(b_hh2[2 * H2:], 1),
        cb1x8=cols(AS * np.asarray(bias1, np.float32), 2),
        cb2x8=cols(AS * np.asarray(bias2, np.float32), 1),
        linb=np.asarray(lin_b, np.float32).reshape(1, c.OUT),
    )

    lw = np.zeros((c.OUT, c.NPAD, c.H2), np.float32)
    lw[:, :c.N, :] = lin_W.reshape(c.OUT, c.N, c.H2)

    in_maps = []
    for k in range(c.NC):
        sl = slice(k * c.NOWN, (k + 1) * c.NOWN)
        m = dict(common)
        m["xT"] = np.ascontiguousarray(xp[:, sl, :].transpose(0, 2, 1)).astype(E4M3)
        mk = M[sl, :].T.reshape(c.CT, P, c.NOWN)            # (ct, p, n)
        m["mTt"] = (MS * np.ascontiguousarray(mk.transpose(1, 0, 2))
                    ).reshape(P, -1).astype(E4M3)
        # lin pairs: lwp[f, j, r, o] = LS * lin_W[o, own_node(j + r*640), f]
        lwk = lw[:, sl, :]                                   # (100, 1280, 128)
        lwr = np.zeros((P, c.NPAIR, 2, OPAD), np.float32)
        lwr[:, :, 0, :c.OUT] = LS * lwk[:, :c.NPAIR, :].transpose(2, 1, 0)
        lwr[:, :, 1, :c.OUT] = LS * lwk[:, c.NPAIR:, :].transpose(2, 1, 0)
        m["lwp"] = lwr.reshape(P, -1).astype(E4M3)
        in_maps.append(m)
    return in_maps


_CACHE = {}


def _get_built(key, cfg):
    if key not in _CACHE:
        _CACHE[key] = build(cfg)
    return _CACHE[key]


def run(cfg: Cfg, inputs, trace=False):
    _install_ntff_hook()
    nc = _get_built(("cfg", cfg.T, cfg.N), cfg)
    in_maps = host_prep(cfg, **inputs)
    res = run_bass_kernel_spmd(nc, in_maps, core_ids=list(range(cfg.NC)), trace=trace)
    return res


def kernel(**inputs) -> np.ndarray:
    cfg = Cfg()
    res = run(cfg, inputs)
    return np.asarray(res.results[0]["out"], np.float32)
